# revision 9
# baseline (speedup 1.0000x reference)
"""nn_ExplicitSVDBlock on 8 Trainium2 NeuronCores (Bass/Tile).

Sharding: data-parallel on batch (cores 0-3 -> batch 0, cores 4-7 -> batch 1),
sequence-parallel within each 4-core group (512 tokens per core end-to-end).
Attention needs full-sequence K/V, obtained with one AllGather of the RoPE'd
K^T and of V (augmented with a per-head always-one column that yields the
softmax denominator for free) inside each 4-core replica group.

All GEMMs run in bf16 with f32 PSUM accumulation. LayerNorm gains/biases, the
QKV/out-proj/FFN biases and the RoPE rotation are folded into the weight
matrices on the host, so the device does matmuls, one tensor_scalar per LN
tile, three elementwise ops per RoPE tensor, exp, GEGLU and residual adds.
"""

import copy
import os
import sys

import numpy as np

for _p in ("/opt/trn_rl_repo", "/root/.axon_site/_ro/trn_rl_repo"):
    if os.path.isdir(_p) and _p not in sys.path:
        sys.path.append(_p)

import ml_dtypes

import concourse.bass as bass
import concourse.mybir as mybir
import concourse.tile as tile
from concourse import bass_utils

# problem constants
B, M, D, H = 2, 2048, 1024, 16
DH = D // H
R = 512           # attention factor rank
FFN_D = 2048
RF = 1024         # ffn factor rank
ROPE_BASE = 10000.0
LN_EPS = 1e-5
N_CORES = 8
S = (B * M) // N_CORES      # 512 tokens per core
G = N_CORES // B            # 4 cores per batch group
SCALE = 1.0 / np.sqrt(DH)
NV = H * (DH + 1)           # v width incl. per-head ones column
KT = M // 128               # key tiles
QT = S // 128               # query/token tiles per core

BF16 = ml_dtypes.bfloat16
F32 = np.float32
_bf = mybir.dt.bfloat16
_f32 = mybir.dt.float32


def _split_multiwait_ctrl(nc):
    """This walrus's codegen accepts a single sync-wait per instruction for
    several opcodes (Drain/NoOp/DMA); hoist extra waits onto preceding
    single-wait NoOps on the same engine (sequencers execute in order, so a
    chain of single-wait NoOps is equivalent to one multi-wait)."""
    for bb in nc.main_func.blocks:
        out, changed = [], False
        for ins in bb.instructions:
            w = list(ins.sync_info.on_wait) if ins.sync_info and ins.sync_info.on_wait else []
            if len(w) > 1:
                changed = True
                for k, wait in enumerate(w[:-1]):
                    n = mybir.InstNoOp(
                        name=f"{ins.name}sw{k}",
                        sync_info=mybir.SyncInfo(on_wait=[wait], on_update=[]),
                        bass_nofuse=True,
                        engine=ins.engine,
                    )
                    out.append(n)
                ins.sync_info.on_wait = [w[-1]]
            out.append(ins)
        if changed:
            try:
                bb.instructions = out
            except Exception:
                bb.instructions.clear()
                bb.instructions.extend(out)
    return nc


# ----------------------------------------------------------------------------
# host-side weight preprocessing
# ----------------------------------------------------------------------------

def _rope_tables():
    inv_freq = 1.0 / (ROPE_BASE ** (np.arange(0, DH, 2, dtype=np.float64) / DH))
    pos = np.arange(M, dtype=np.float64)[:, None] * inv_freq[None, :]
    emb = np.concatenate((pos, pos), axis=1)
    return np.cos(emb), np.sin(emb)                     # [M, 64]


def _rot_cols(V):
    """rotate_half applied to the head-dim output columns of V ([*, D])."""
    Vr = np.empty_like(V)
    for h in range(H):
        c = h * DH
        Vr[:, c:c + 32] = -V[:, c + 32:c + 64]
        Vr[:, c + 32:c + 64] = V[:, c:c + 32]
    return Vr


def _prep(inputs):
    f = lambda t: np.asarray(t, dtype=np.float64)
    ln1_g, ln1_b = f(inputs["ln1_g"]), f(inputs["ln1_b"])
    ln2_g, ln2_b = f(inputs["ln2_g"]), f(inputs["ln2_b"])
    w = {}

    def aug(Vmat, bias):
        return np.concatenate([Vmat, bias[None, :]], axis=0)

    for name in ("q", "k"):
        U, V, b = f(inputs[f"U{name}"]), f(inputs[f"V{name}"]), f(inputs[f"b{name}"])
        Va = aug(V, b + (ln1_b @ U) @ V)
        w[f"U{name}"] = (ln1_g[:, None] * U).astype(BF16)
        w[f"V{name}a"] = Va.astype(BF16)
        w[f"V{name}r"] = _rot_cols(Va).astype(BF16)

    Uv, Vv, bv = f(inputs["Uv"]), f(inputs["Vv"]), f(inputs["bv"])
    Vva = aug(Vv, bv + (ln1_b @ Uv) @ Vv)               # [513, 1024]
    Vva_i = np.zeros((R + 1, NV), dtype=np.float64)
    for h in range(H):
        Vva_i[:, h * 65:h * 65 + 64] = Vva[:, h * DH:(h + 1) * DH]
        Vva_i[R, h * 65 + 64] = 1.0
    w["Uv"] = (ln1_g[:, None] * Uv).astype(BF16)
    w["Vva"] = Vva_i.astype(BF16)

    Wo, bo = f(inputs["Wo"]), f(inputs["bo"])
    w["WoTa"] = np.concatenate([Wo.T, bo[None, :]], axis=0).astype(BF16)

    U1, V1, b1 = f(inputs["U1"]), f(inputs["V1"]), f(inputs["b1"])
    U2, V2, b2 = f(inputs["U2"]), f(inputs["V2"]), f(inputs["b2"])
    b1p = b1 + (ln2_b @ U1) @ V1
    w["U1"] = (ln2_g[:, None] * U1).astype(BF16)
    # V1 repacked pair-major: pair p holds z1 block p and z2 block p
    V1p = np.empty((FFN_D // 128, RF, 256), dtype=np.float64)
    for p in range(FFN_D // 128):
        V1p[p, :, 0:128] = V1[:, p * 128:(p + 1) * 128]
        V1p[p, :, 128:256] = V1[:, FFN_D + p * 128:FFN_D + (p + 1) * 128]
    w["V1p"] = V1p.reshape((FFN_D // 128) * RF, 256).astype(BF16)
    w["b1_sb"] = b1p.reshape(2 * FFN_D // 128, 128).T.copy().astype(F32)
    w["U2"] = U2.astype(BF16)
    w["V2a"] = np.concatenate([V2, b2[None, :]], axis=0).astype(BF16)
    w["ident"] = np.eye(128, dtype=BF16)

    cos, sin = _rope_tables()
    x = np.asarray(inputs["hidden_states"], dtype=F32)
    per_core = []
    for core in range(N_CORES):
        g, s = divmod(core, G)
        p0 = s * S
        im = {"x": np.ascontiguousarray(x[g, p0:p0 + S, :]),
              "cos_sb": np.tile(cos.T[:, p0:p0 + S], (2, 1)).astype(F32),
              "sin_sb": np.tile(sin.T[:, p0:p0 + S], (2, 1)).astype(F32)}
        im.update(w)
        per_core.append(im)
    return per_core


# ----------------------------------------------------------------------------
# device kernel
# ----------------------------------------------------------------------------

def build_module(n_iters=1, stage=99):
    nc = bass.Bass("TRN2", target_bir_lowering=False, debug=False,
                   num_devices=N_CORES)

    din = {}
    def inp(name, shape, dt=_bf):
        din[name] = nc.dram_tensor(name, list(shape), dt, kind="ExternalInput")

    inp("x", [S, D], _f32)
    inp("cos_sb", [128, S], _f32)
    inp("sin_sb", [128, S], _f32)
    for n in ("Uq", "Uk", "Uv"):
        inp(n, [D, R])
    for n in ("Vqa", "Vqr", "Vka", "Vkr"):
        inp(n, [R + 1, D])
    inp("Vva", [R + 1, NV])
    inp("WoTa", [D + 1, D])
    inp("U1", [D, RF])
    inp("V1p", [(FFN_D // 128) * RF, 256])
    inp("b1_sb", [128, 2 * FFN_D // 128], _f32)
    inp("U2", [FFN_D, RF])
    inp("V2a", [RF + 1, D])
    inp("ident", [128, 128])

    y_out = nc.dram_tensor("y", [S, D], _f32, kind="ExternalOutput")
    dbg_outs = []

    def dbg(name, shape, dt=_f32):
        t = nc.dram_tensor(name, list(shape), dt, kind="ExternalOutput")
        dbg_outs.append(name)
        return t

    groups = [[0, 1, 2, 3], [4, 5, 6, 7]]

    with tile.TileContext(nc) as tc:
        import contextlib
        with contextlib.ExitStack() as octx:
            dram = octx.enter_context(tc.tile_pool(name="dram", bufs=1, space="DRAM"))
            const_p = octx.enter_context(tc.tile_pool(name="const", bufs=1))

            kt_in = dram.tile([D, S], _bf)
            kt_out = dram.tile([G * D, S], _bf)
            v_in = dram.tile([S, NV], _bf)
            v_out = dram.tile([G * S, NV], _bf)

            def cload(name, shape, dt=_f32):
                t = const_p.tile(list(shape), dt, tag=name)
                nc.sync.dma_start(t[:], din[name].ap())
                return t

            C = dict(
                ident=cload("ident", [128, 128], _bf),
                cos_sb=cload("cos_sb", [128, S]),
                sin_sb=cload("sin_sb", [128, S]),
                b1_sb=cload("b1_sb", [128, 2 * FFN_D // 128]),
            )
            ones_t = const_p.tile([1, S], _bf, tag="ones")
            nc.vector.memset(ones_t[:], 1.0)
            eps_tile = const_p.tile([128, 1], _f32, tag="eps")
            nc.vector.memset(eps_tile[:], LN_EPS)
            C["ones"] = ones_t
            C["eps"] = eps_tile
            cc = dict(kt_in=kt_in, kt_out=kt_out, v_in=v_in, v_out=v_out)

            for _ in range(n_iters):
                _iter(nc, tc, din, y_out, groups, stage, dbg, C, cc)

    _split_multiwait_ctrl(nc)
    return nc, dbg_outs


def _wload(nc, pool, din, name, ndim):
    """[K, N] dram weight -> sbuf [128, K/128, N] (lhsT K-tiles)."""
    kdim = din[name].shape[0] // 128 * 128
    t = pool.tile([128, (kdim // 128) * ndim], _bf, tag=name)
    ap3 = din[name].ap()[0:kdim, :].rearrange("(kt p) n -> p kt n", p=128)
    nc.sync.dma_start(t[:].rearrange("p (kt n) -> p kt n", n=ndim), ap3)
    return t[:].rearrange("p (kt n) -> p kt n", n=ndim)


def _wrow(nc, pool, din, name, ndim):
    """last (bias) row of an augmented weight."""
    t = pool.tile([1, ndim], _bf, tag=name + "_r")
    off = din[name].shape[0] // 128 * 128
    nc.sync.dma_start(t[:], din[name].ap()[off:off + 1, :])
    return t


def _ln_tile(nc, tmp_p, pool, xt, eps_t, t, tag):
    x3 = xt[:].rearrange("p (sg d) -> p sg d", sg=2)
    stats = tmp_p.tile([128, 2, 6], _f32, tag="bn_stats")
    for sg in range(2):
        nc.vector.bn_stats(stats[:, sg, :], x3[:, sg, :])
    mv = tmp_p.tile([128, 2], _f32, tag="bn_aggr")
    nc.vector.bn_aggr(mv[:], stats[:])
    rstd = tmp_p.tile([128, 1], _f32, tag="rstd")
    nc.scalar.activation(rstd[:], mv[:, 1:2], mybir.ActivationFunctionType.Sqrt,
                         bias=eps_t[:])
    nc.vector.reciprocal(rstd[:], rstd[:])
    xn = pool.tile([128, D], _bf, tag=f"{tag}{t}")
    nc.vector.tensor_scalar(xn[:], xt[:], mv[:, 0:1], rstd[:],
                            mybir.AluOpType.subtract, mybir.AluOpType.mult)
    return xn


def _transpose(nc, tc, pool, ident, tiles_pd, tag):
    """[QT x (128, D')] token-major bf16 -> [D'/128 x (128, 128*QT)] transposed."""
    nqt = len(tiles_pd)
    Dp = tiles_pd[0].shape[-1]
    outs = [pool.tile([128, 128 * nqt], _bf, tag=f"{tag}{c}", name=f"{tag}{c}")
            for c in range(Dp // 128)]
    with tc.tile_pool(name=f"ps_{tag}", bufs=2, space="PSUM") as ps_tr:
        for r in range(nqt):
            for c in range(Dp // 128):
                p = ps_tr.tile([128, 128], _bf, tag="tr")
                nc.tensor.transpose(p[:], tiles_pd[r][:, c * 128:(c + 1) * 128],
                                    ident[:])
                nc.scalar.copy(outs[c][:, r * 128:(r + 1) * 128], p[:])
    return outs


def _iter(nc, tc, din, y_out, groups, stage, dbg, C, cc):
    import contextlib
    ident, cos_sb, sin_sb = C["ident"], C["cos_sb"], C["sin_sb"]
    b1_sb, ones, eps_t = C["b1_sb"], C["ones"], C["eps"]
    AF = mybir.ActivationFunctionType

    with contextlib.ExitStack() as ictx:
        ps = ictx.enter_context(tc.tile_pool(name="ps", bufs=3, space="PSUM"))
        tmp_p = ictx.enter_context(tc.tile_pool(name="tmps", bufs=2))
        fpool = ictx.enter_context(tc.tile_pool(name="actFFN", bufs=1))
        attctx = ictx.enter_context(contextlib.ExitStack())
        xpool = attctx.enter_context(tc.tile_pool(name="xres", bufs=1))
        qpool = attctx.enter_context(tc.tile_pool(name="qT", bufs=1))

        def mm_chain(out_psum, lhs_tiles_fn, rhs_fn, nk, aug=None):
            for k in range(nk):
                nc.tensor.matmul(out_psum, lhs_tiles_fn(k), rhs_fn(k),
                                 start=(k == 0), stop=(k == nk - 1 and aug is None))
            if aug is not None:
                nc.tensor.matmul(out_psum, aug[0], aug[1], start=False, stop=True)

        # ---------------- phase A+B+C+D: projections ----------------------
        with contextlib.ExitStack() as pctx:
            wq = pctx.enter_context(tc.tile_pool(name="wqkv", bufs=1))
            ab = pctx.enter_context(tc.tile_pool(name="actAB", bufs=1))

            Uq = _wload(nc, wq, din, "Uq", R)
            Uk = _wload(nc, wq, din, "Uk", R)
            Uv = _wload(nc, wq, din, "Uv", R)
            Vqa, Vqa_r = _wload(nc, wq, din, "Vqa", D), _wrow(nc, wq, din, "Vqa", D)
            Vqr, Vqr_r = _wload(nc, wq, din, "Vqr", D), _wrow(nc, wq, din, "Vqr", D)
            Vka, Vka_r = _wload(nc, wq, din, "Vka", D), _wrow(nc, wq, din, "Vka", D)
            Vkr, Vkr_r = _wload(nc, wq, din, "Vkr", D), _wrow(nc, wq, din, "Vkr", D)
            Vva, Vva_r = _wload(nc, wq, din, "Vva", NV), _wrow(nc, wq, din, "Vva", NV)

            x_ap3 = din["x"].ap().rearrange("(t p) dcol -> t p dcol", p=128)
            x_sb, xn_sb = [], []
            for t in range(QT):
                xt = xpool.tile([128, D], _f32, tag=f"x{t}")
                nc.sync.dma_start(xt[:], x_ap3[t])
                x_sb.append(xt)
                xn_sb.append(_ln_tile(nc, tmp_p, ab, xt, eps_t, t, "xn"))

            if stage == 0:
                o = dbg("xn_dbg", [S, D], _bf)
                for t in range(QT):
                    nc.sync.dma_start(o.ap()[t * 128:(t + 1) * 128, :], xn_sb[t][:])
                return

            xnT = _transpose(nc, tc, ab, ident, xn_sb, "xnT")

            def rank_gemm(U, tag):
                outs = []
                for m in range(R // 128):
                    p = ps.tile([128, S], _f32, tag="mm")
                    mm_chain(p[:], lambda k, m=m: U[:, k, m * 128:(m + 1) * 128],
                             lambda k: xnT[k][:], D // 128)
                    t = ab.tile([128, S], _bf, tag=f"{tag}{m}")
                    nc.scalar.copy(t[:], p[:])
                    outs.append(t)
                return outs

            tq, tk, tv = rank_gemm(Uq, "tq"), rank_gemm(Uk, "tk"), rank_gemm(Uv, "tv")

            if stage == 1:
                o = dbg("tq_dbg", [R, S], _bf)
                for m in range(R // 128):
                    nc.sync.dma_start(o.ap()[m * 128:(m + 1) * 128, :], tq[m][:])
                return

            def head_gemm(Va, Va_r, rot, rot_r, tmat, pool, tag):
                outs = []
                for m in range(D // 128):
                    p = ps.tile([128, S], _f32, tag="mm")
                    mm_chain(p[:], lambda k, m=m: Va[:, k, m * 128:(m + 1) * 128],
                             lambda k: tmat[k][:], R // 128,
                             aug=(Va_r[:, m * 128:(m + 1) * 128], ones[:]))
                    p2 = ps.tile([128, S], _f32, tag="mm")
                    mm_chain(p2[:], lambda k, m=m: rot[:, k, m * 128:(m + 1) * 128],
                             lambda k: tmat[k][:], R // 128,
                             aug=(rot_r[:, m * 128:(m + 1) * 128], ones[:]))
                    a = tmp_p.tile([128, S], _f32, tag="rope_a")
                    nc.vector.tensor_mul(a[:], p[:], cos_sb[:])
                    b = tmp_p.tile([128, S], _f32, tag="rope_b")
                    nc.vector.tensor_mul(b[:], p2[:], sin_sb[:])
                    t = pool.tile([128, S], _bf, tag=f"{tag}{m}")
                    nc.vector.tensor_add(t[:], a[:], b[:])
                    outs.append(t)
                return outs

            qT = head_gemm(Vqa, Vqa_r, Vqr, Vqr_r, tq, qpool, "qT")
            kTl = head_gemm(Vka, Vka_r, Vkr, Vkr_r, tk, ab, "kT")

            v_loc = []
            for m in range(QT):
                t = ab.tile([128, NV], _bf, tag=f"v{m}")
                for i, c0 in enumerate(range(0, NV, 512)):
                    cw = min(512, NV - c0)
                    p = ps.tile([128, 512], _f32, tag="mm")
                    mm_chain(p[:, :cw],
                             lambda k, m=m: tv[k][:, m * 128:(m + 1) * 128],
                             lambda k, c0=c0, cw=cw: Vva[:, k, c0:c0 + cw],
                             R // 128,
                             aug=(ones[:, m * 128:(m + 1) * 128],
                                  Vva_r[:, c0:c0 + cw]))
                    nc.scalar.copy(t[:, c0:c0 + cw], p[:, :cw])
                v_loc.append(t)

            if stage == 2:
                o, o2, o3 = dbg("qT_dbg", [D, S], _bf), dbg("kT_dbg", [D, S], _bf), dbg("v_dbg", [S, NV], _bf)
                for m in range(D // 128):
                    nc.sync.dma_start(o.ap()[m * 128:(m + 1) * 128, :], qT[m][:])
                    nc.sync.dma_start(o2.ap()[m * 128:(m + 1) * 128, :], kTl[m][:])
                for m in range(QT):
                    nc.sync.dma_start(o3.ap()[m * 128:(m + 1) * 128, :], v_loc[m][:])
                return

            # bounce out for the collectives
            for m in range(D // 128):
                nc.gpsimd.dma_start(cc["kt_in"][m * 128:(m + 1) * 128, :], kTl[m][:])
            for m in range(QT):
                nc.gpsimd.dma_start(cc["v_in"][m * 128:(m + 1) * 128, :], v_loc[m][:])

        # ---------------- phase E: allgather ------------------------------
        nc.gpsimd.collective_compute(
            "AllGather", mybir.AluOpType.bypass,
            ins=[cc["kt_in"].opt()], outs=[cc["kt_out"].opt()], replica_groups=groups)
        nc.gpsimd.collective_compute(
            "AllGather", mybir.AluOpType.bypass,
            ins=[cc["v_in"].opt()], outs=[cc["v_out"].opt()], replica_groups=groups)

        with contextlib.ExitStack() as actx:
            at = actx.enter_context(tc.tile_pool(name="actATT", bufs=1))
            wmid = actx.enter_context(tc.tile_pool(name="wmid", bufs=1))
            WoTa, WoTa_r = _wload(nc, wmid, din, "WoTa", D), _wrow(nc, wmid, din, "WoTa", D)

            kT = []
            for m in range(D // 128):
                t = at.tile([128, M], _bf, tag=f"kTf{m}")
                for j in range(G):
                    nc.sync.dma_start(t[:, j * S:(j + 1) * S],
                                      cc["kt_out"][j * D + m * 128:j * D + (m + 1) * 128, :])
                kT.append(t)
            vF = []
            for m in range(KT):
                t = at.tile([128, NV], _bf, tag=f"vF{m}")
                nc.sync.dma_start(t[:], cc["v_out"][m * 128:(m + 1) * 128, :])
                vF.append(t)

            if stage == 3:
                o, o2 = dbg("kTf_dbg", [D, M], _bf), dbg("vF_dbg", [M, NV], _bf)
                for m in range(D // 128):
                    nc.sync.dma_start(o.ap()[m * 128:(m + 1) * 128, :], kT[m][:])
                for m in range(KT):
                    nc.sync.dma_start(o2.ap()[m * 128:(m + 1) * 128, :], vF[m][:])
                return

            # ---------------- phase F: attention --------------------------
            attn = [at.tile([128, D], _bf, tag=f"attn{t}", name=f"attn{t}")
                    for t in range(QT)]
            with tc.tile_pool(name="ps_av", bufs=1, space="PSUM") as ps_av, \
                 tc.tile_pool(name="exp_p", bufs=4) as exp_p:
                for h in range(H):
                    hd = h * DH
                    po = hd % 128
                    av = [ps_av.tile([128, DH + 1], _f32, tag=f"av{qt}",
                                     name=f"av{h}_{qt}")
                          for qt in range(QT)]
                    for kt in range(KT):
                        st = ps.tile([128, S], _f32, tag="mm")
                        nc.tensor.matmul(
                            st[:],
                            kT[hd // 128][po:po + DH, kt * 128:(kt + 1) * 128],
                            qT[hd // 128][po:po + DH, :],
                            start=True, stop=True)
                        ex = exp_p.tile([128, S], _bf, tag="exp")
                        nc.scalar.activation(ex[:], st[:], AF.Exp, scale=float(SCALE))
                        for qt in range(QT):
                            nc.tensor.matmul(av[qt][:],
                                             ex[:, qt * 128:(qt + 1) * 128],
                                             vF[kt][:, h * 65:h * 65 + 65],
                                             start=(kt == 0), stop=(kt == KT - 1))
                    for qt in range(QT):
                        rec = tmp_p.tile([128, 1], _f32, tag="rec")
                        nc.vector.reciprocal(rec[:], av[qt][:, DH:DH + 1])
                        nc.scalar.activation(attn[qt][:, hd:hd + DH],
                                             av[qt][:, 0:DH], AF.Copy, scale=rec[:])

            if stage == 4:
                o = dbg("attn_dbg", [S, D], _bf)
                for t in range(QT):
                    nc.sync.dma_start(o.ap()[t * 128:(t + 1) * 128, :], attn[t][:])
                return

            # ---------------- phase G: out-proj + residual ----------------
            aoT = _transpose(nc, tc, at, ident, attn, "aoT")
            x2_sb, xn2_sb = [], []
            for t in range(QT):
                x2 = fpool.tile([128, D], _f32, tag=f"x2_{t}")
                for c in range(D // 512):
                    p = ps.tile([128, 512], _f32, tag="mm")
                    mm_chain(p[:],
                             lambda k, t=t: aoT[k][:, t * 128:(t + 1) * 128],
                             lambda k, c=c: WoTa[:, k, c * 512:(c + 1) * 512],
                             D // 128,
                             aug=(ones[:, t * 128:(t + 1) * 128],
                                  WoTa_r[:, c * 512:(c + 1) * 512]))
                    nc.vector.tensor_add(x2[:, c * 512:(c + 1) * 512], p[:],
                                         x_sb[t][:, c * 512:(c + 1) * 512])
                x2_sb.append(x2)
                xn2_sb.append(_ln_tile(nc, tmp_p, fpool, x2, eps_t, t, "xn2"))

            if stage == 5:
                o = dbg("x2_dbg", [S, D])
                for t in range(QT):
                    nc.sync.dma_start(o.ap()[t * 128:(t + 1) * 128, :], x2_sb[t][:])
                return

        # ---------------- phase H/I: FFN ----------------------------------
        attctx.close()
        with contextlib.ExitStack() as fctx:
            fp = fctx.enter_context(tc.tile_pool(name="actF2", bufs=1))
            wffn = fctx.enter_context(tc.tile_pool(name="wffn", bufs=1))
            U1 = _wload(nc, wffn, din, "U1", RF)
            U2 = _wload(nc, wffn, din, "U2", RF)
            V2a, V2a_r = _wload(nc, wffn, din, "V2a", D), _wrow(nc, wffn, din, "V2a", D)

            xn2T = _transpose(nc, tc, fp, ident, xn2_sb, "xn2T")

            t1T = []
            for m in range(RF // 128):
                p = ps.tile([128, S], _f32, tag="mm")
                mm_chain(p[:], lambda k, m=m: U1[:, k, m * 128:(m + 1) * 128],
                         lambda k: xn2T[k][:], D // 128)
                t = fp.tile([128, S], _bf, tag=f"t1T{m}")
                nc.scalar.copy(t[:], p[:])
                t1T.append(t)

            hT = []
            with tc.tile_pool(name="v1s", bufs=3) as v1sp:
                v1_ap = din["V1p"].ap().rearrange("(pr kt p) n -> pr p kt n", p=128,
                                                  kt=RF // 128)
                for pr in range(FFN_D // 128):
                    v1c = v1sp.tile([128, (RF // 128) * 256], _bf, tag="v1c")
                    v1c3 = v1c[:].rearrange("p (kt n) -> p kt n", n=256)
                    nc.sync.dma_start(v1c3, v1_ap[pr])
                    p1 = ps.tile([128, S], _f32, tag="mm")
                    mm_chain(p1[:], lambda k: v1c3[:, k, 0:128],
                             lambda k: t1T[k][:], RF // 128)
                    p2 = ps.tile([128, S], _f32, tag="mm")
                    mm_chain(p2[:], lambda k: v1c3[:, k, 128:256],
                             lambda k: t1T[k][:], RF // 128)
                    ge = tmp_p.tile([128, S], _f32, tag="gelu")
                    nc.scalar.activation(ge[:], p1[:], AF.Gelu_apprx_tanh,
                                         bias=b1_sb[:, pr:pr + 1])
                    z2 = tmp_p.tile([128, S], _f32, tag="z2b")
                    nc.vector.tensor_scalar_add(z2[:], p2[:],
                                                b1_sb[:, 16 + pr:17 + pr])
                    t = fp.tile([128, S], _bf, tag=f"hT{pr}")
                    nc.vector.tensor_mul(t[:], ge[:], z2[:])
                    hT.append(t)

            if stage == 6:
                o = dbg("hT_dbg", [FFN_D, S], _bf)
                for m in range(FFN_D // 128):
                    nc.sync.dma_start(o.ap()[m * 128:(m + 1) * 128, :], hT[m][:])
                return

            t2T = []
            for m in range(RF // 128):
                p = ps.tile([128, S], _f32, tag="mm")
                mm_chain(p[:], lambda k, m=m: U2[:, k, m * 128:(m + 1) * 128],
                         lambda k: hT[k][:], FFN_D // 128)
                t = fp.tile([128, S], _bf, tag=f"t2T{m}")
                nc.scalar.copy(t[:], p[:])
                t2T.append(t)

            for t in range(QT):
                yt = tmp_p.tile([128, D], _f32, tag="y_t")
                for c in range(D // 512):
                    p = ps.tile([128, 512], _f32, tag="mm")
                    mm_chain(p[:],
                             lambda k, t=t: t2T[k][:, t * 128:(t + 1) * 128],
                             lambda k, c=c: V2a[:, k, c * 512:(c + 1) * 512],
                             RF // 128,
                             aug=(ones[:, t * 128:(t + 1) * 128],
                                  V2a_r[:, c * 512:(c + 1) * 512]))
                    nc.vector.tensor_add(yt[:, c * 512:(c + 1) * 512], p[:],
                                         x2_sb[t][:, c * 512:(c + 1) * 512])
                nc.sync.dma_start(y_out.ap()[t * 128:(t + 1) * 128, :], yt[:])


# ----------------------------------------------------------------------------
# public entry point
# ----------------------------------------------------------------------------

_CACHE = {}


def _run(per_core, n_iters=1, stage=99):
    key = (n_iters, stage)
    if key not in _CACHE:
        _CACHE[key] = build_module(n_iters=n_iters, stage=stage)
    nc, dbg_outs = _CACHE[key]
    res = bass_utils.run_bass_kernel_spmd(
        nc, per_core, core_ids=list(range(N_CORES)), trace=False)
    return res, dbg_outs


def kernel(**inputs):
    per_core = _prep(inputs)
    res, _ = _run(per_core, n_iters=1, stage=99)
    y = np.empty((B, M, D), dtype=np.float32)
    for core in range(N_CORES):
        g, s = divmod(core, G)
        y[g, s * S:(s + 1) * S, :] = res.results[core]["y"]
    return y
